# revision 3
# baseline (speedup 1.0000x reference)
"""Block-scaled fp8 ColumnParallelLinear for Trainium2 — w-stationary fp8
DoubleRow GEMM with exact top-N partial-residual correction.
Measured: 298.5us, rel err 1.922e-2 (baseline 350.7us / 1.447e-2).

Reference semantics (per token m, output o):
    x_scale[m] = max(|x[m, :]|) / 448
    x_q[m, k]  = e4m3fn_round(x[m, k] / x_scale[m])
    w_deq[o,k] = e4m3(w)[o, k] * s[o//128, k//128]
    y[m, o]    = x_scale[m] * sum_k x_q[m, k] * w_deq[o, k]

Approach (8 M-shards, O replicated -> one SPMD program):
  - Host does ALL quantization. x on the exact half grid (224/amax,
    TRN e4m3 max 240): device bytes = reference fp8 values / 2, the
    factor folds into the output scale sc[m] = t * amax[m]/224, so the
    x side contributes ZERO error vs the reference.
  - w1 = trn_fp8(w_deq/t) with one global t; residual w2 =
    trn_fp8(w_deq/t - w1) applied only on the top-N_COVER (128o x
    256k) blocks by residual energy (N=256 of 1024 -> rel err
    1.922e-2, gate 2e-2; host model matches hw to <0.1%) at 1.25x the
    pure-fp8 FLOPs. 1280 matmuls x 216ns = the 276.5us tensor floor.
  - Device: W-STATIONARY fp8 DoubleRow matmuls. Stationary =
    w tile [128k, 2, 128o]; moving = x [128k, 2, 512m] (the whole
    M-shard per instr; moving max 1024 = 2*512). Out psum
    [128o, 512m] = 1 bank. Every matmul (main or residual) is a full
    512-col instr (213ns) > LDWEIGHTS (135ns), so the per-matmul
    LDWEIGHTS that legalization always inserts stays hidden, and the
    residual needs no pair-merging/padding: exactly the selected
    blocks are covered, one instr each.
  - Pair-interleaved compute: the two o-blocks of an ob-pair sweep j
    together (two live psum banks), so each x j-chunk feeds two
    matmuls on arrival — halves the startup bandwidth demand. Per ob:
    16 main DR matmuls + one per covered (ob, j); residuals sit at
    the end of the sweep (w2 off the critical DMA prefix). DVE
    applies the per-token scale (free-dim vector, broadcast across
    partitions); output DMA (scalar engine) writes yt[o, m]
    transposed, host transposes back. Last ob drains in halves.
  - One sync-engine DMA FIFO in exact need order (same-queue FIFO =
    implicit bandwidth priority; parallel queues contend): 4-j
    startup chunks for the lightest-residual pair first, then 1MiB
    per-pair slabs, 5-deep rotation. dma_start costs ~0.7us of issue
    time each, so startup uses few, large transfers.
  - 28 warm-up matmuls on zeroed scratch ramp the PE clock out of its
    low p-state during the ~5us startup DMA wait; without them the
    first ~10 real matmuls run ~2x slow.
"""

import numpy as np
import ml_dtypes

import concourse.bass as bass
import concourse.mybir as mybir
from concourse import bacc
from concourse.tile import TileContext

FP8_MAX = 448.0
HALF_MAX = 224.0
P = 128
BLOCK = 128

M_FULL, K_FULL, O_FULL = 4096, 4096, 8192
N_CORES = 8
M_LOC = M_FULL // N_CORES  # 512 tokens per core; O replicated

KTP_N = K_FULL // (2 * P)  # 16 k-pair tiles
OB_N = O_FULL // P  # 64 o-blocks
N_COVER = 256  # top-N residual blocks of OB_N*KTP_N=1024

DR = mybir.MatmulPerfMode.DoubleRow


def _select(resid):
    """sel[ob] = ascending list of covered j for the top-N_COVER blocks."""
    E = (resid.astype(np.float32) ** 2).reshape(OB_N, P, KTP_N, 2 * P).sum(axis=(1, 3))
    flat = E.flatten()
    order = np.argsort(flat)[::-1][:N_COVER]
    mask = np.zeros(flat.size, bool)
    mask[order] = True
    mask = mask.reshape(OB_N, KTP_N)
    return [sorted(np.nonzero(mask[ob])[0].tolist()) for ob in range(OB_N)]


def build_bass(sel):
    nc = bacc.Bacc()
    f32 = mybir.dt.float32
    fp8 = mybir.dt.float8e4

    nsel = [len(s) for s in sel]
    total_cols = sum(nsel) * P

    xq = nc.declare_dram_parameter("xq", [P, KTP_N, 2, M_LOC], fp8, isOutput=False)
    # pair-major: [pair, p, half, j, u, ol] so one pair DMA is contiguous
    w1t = nc.declare_dram_parameter(
        "w1t", [OB_N // 2, P, 2, KTP_N, 2, P], fp8, isOutput=False
    )
    w2t = nc.declare_dram_parameter(
        "w2t", [P, 2, max(total_cols, P)], fp8, isOutput=False
    )
    scb = nc.declare_dram_parameter("scb", [P, M_LOC], f32, isOutput=False)
    yt = nc.declare_dram_parameter("yt", [O_FULL, M_LOC], f32, isOutput=True)

    # w2 column offset (in o-blocks) per ob, in (ob, j) packing order
    off = np.cumsum([0] + nsel)

    with TileContext(nc) as tc:
        with (
            tc.tile_pool(name="xp", bufs=1) as xp,
            tc.tile_pool(name="wp", bufs=32) as wp,
            tc.tile_pool(name="w2p", bufs=32) as w2p,
            tc.tile_pool(name="op", bufs=6) as op,
            tc.tile_pool(name="mm", bufs=8, space="PSUM") as pp,
        ):
            xs = xp.tile([P, KTP_N, 2, M_LOC], fp8)
            scs = xp.tile([P, M_LOC], f32)

            # Warm-up: dependency-free matmuls on (uninitialized) scratch
            # start right after the tensor preamble, ramping the PE clock
            # through the ~5us DMA wait so real matmuls run at full speed
            # from the first one. Results land in an otherwise-unused psum
            # bank and are never read.
            warm = xp.tile([P, 2, P], fp8)
            psw = pp.tile([P, M_LOC], f32, tag="warm", bufs=1, name="ps_warm")
            nc.scalar.memzero(warm[:])
            for i in range(28):
                nc.tensor.matmul(
                    psw[:, 0:P],
                    warm[:],
                    warm[:],
                    start=True,
                    stop=True,
                    perf_mode=DR,
                )

            OBP = OB_N // 2  # 32 ob-pairs; one w1/w2 DMA per pair
            # process the lightest-residual pair first: smallest critical
            # startup prefix (its w2 slab may even be empty)
            p0 = min(range(OBP), key=lambda pr: nsel[2 * pr] + nsel[2 * pr + 1])
            pair_order = [p0] + [pr for pr in range(OBP) if pr != p0]
            wtiles = {}
            rtiles = {}
            for pr in pair_order:
                wtiles[pr] = wp.tile(
                    [P, 2, KTP_N, 2, P], fp8, tag="w1", bufs=5, name=f"w1_{pr}"
                )
                pw = (nsel[2 * pr] + nsel[2 * pr + 1]) * P
                rtiles[pr] = (
                    w2p.tile([P, 2, pw], fp8, tag="w2", bufs=5, name=f"w2_{pr}")
                    if pw
                    else None
                )

            def issue_pair_dma(pr):
                nc.sync.dma_start(out=wtiles[pr][:], in_=w1t[pr])
                if rtiles[pr] is not None:
                    c0, c1 = off[2 * pr] * P, off[2 * pr + 2] * P
                    nc.sync.dma_start(
                        out=rtiles[pr][:], in_=w2t[:, :, c0:c1]
                    )

            # Startup: one sync FIFO stream in exact need order (same-queue
            # FIFO = implicit bandwidth priority; parallel queues contend).
            # Pair 0 is computed ob-interleaved per j, so each 4-j chunk
            # (x 512KB + both-halves w1 256KB) feeds 8 matmuls — delivery
            # and consumption stay matched from the first matmul on.
            for c in range(0, KTP_N, 4):
                nc.sync.dma_start(out=xs[:, c : c + 4], in_=xq[:, c : c + 4])
                nc.sync.dma_start(
                    out=wtiles[p0][:, :, c : c + 4], in_=w1t[p0, :, :, c : c + 4]
                )
            if rtiles[p0] is not None:
                c0, c1 = off[2 * p0] * P, off[2 * p0 + 2] * P
                nc.sync.dma_start(out=rtiles[p0][:], in_=w2t[:, :, c0:c1])
            issue_pair_dma(pair_order[1])
            nc.sync.dma_start(out=scs[:], in_=scb[:])
            for pr in pair_order[2:]:
                issue_pair_dma(pr)

            # Pair-interleaved compute: the two obs of a pair sweep j
            # together (two live psum banks), so each x j-chunk feeds two
            # matmuls as soon as it lands. Per ob: residuals, then the
            # stop=True j15 main; the even ob's DVE+output drain overlaps
            # the odd ob's residual tail.
            for pr in pair_order:
                wt = wtiles[pr]
                obs = (2 * pr, 2 * pr + 1)
                pss = [
                    pp.tile([P, M_LOC], f32, tag="ps", bufs=6, name=f"ps_{ob}")
                    for ob in obs
                ]
                for j in range(KTP_N - 1):
                    for half, ps in enumerate(pss):
                        nc.tensor.matmul(
                            ps[:],
                            wt[:, half, j],
                            xs[:, j],
                            start=(j == 0),
                            stop=False,
                            perf_mode=DR,
                        )
                for half, ps in enumerate(pss):
                    ob = obs[half]
                    ridx = off[ob] - off[2 * pr]
                    for j in sel[ob]:
                        nc.tensor.matmul(
                            ps[:],
                            rtiles[pr][:, :, ridx * P : (ridx + 1) * P],
                            xs[:, j],
                            start=False,
                            stop=False,
                            perf_mode=DR,
                        )
                        ridx += 1
                    nc.tensor.matmul(
                        ps[:],
                        wt[:, half, KTP_N - 1],
                        xs[:, KTP_N - 1],
                        start=False,
                        stop=True,
                        perf_mode=DR,
                    )
                    ot = op.tile([P, M_LOC], f32, tag="out", bufs=4, name=f"o_{ob}")
                    if ob == 2 * pair_order[-1] + 1:
                        # last ob: halve the DVE+DMA drain so the tail
                        # pipelines (DVE h2 overlaps DMA h1)
                        H = M_LOC // 2
                        for h in range(2):
                            sl = slice(h * H, (h + 1) * H)
                            nc.vector.tensor_tensor(
                                ot[:, sl], ps[:, sl], scs[:, sl],
                                mybir.AluOpType.mult,
                            )
                            nc.scalar.dma_start(
                                out=yt[ob * P : (ob + 1) * P, sl], in_=ot[:, sl]
                            )
                    else:
                        nc.vector.tensor_tensor(
                            ot[:], ps[:], scs[:], mybir.AluOpType.mult
                        )
                        nc.scalar.dma_start(
                            out=yt[ob * P : (ob + 1) * P, :], in_=ot[:]
                        )
    return nc


def prep_inputs(x, weight, weight_scale_inv):
    x2d = np.ascontiguousarray(x.reshape(M_FULL, K_FULL).astype(np.float32))
    amax = np.clip(np.abs(x2d).max(axis=1), 1e-12, None)
    xq_all = (x2d * (HALF_MAX / amax)[:, None]).astype(ml_dtypes.float8_e4m3)

    w8 = weight.astype(ml_dtypes.float8_e4m3fn).astype(np.float32)
    s_exp = np.repeat(
        np.repeat(weight_scale_inv.astype(np.float32), BLOCK, 0), BLOCK, 1
    )
    w_deq = w8 * s_exp
    t = float(np.abs(w_deq).max() / HALF_MAX)
    v = w_deq / t
    w1 = v.astype(ml_dtypes.float8_e4m3)
    resid = v - w1.astype(np.float32)
    w2 = resid.astype(ml_dtypes.float8_e4m3)
    sel = _select(resid)

    # w1t[pr, p, half, j, u, ol] from w1[o, k]:
    #   o = pr*256 + half*128 + ol, k = j*256 + u*128 + p
    w1t = np.ascontiguousarray(
        w1.reshape(OB_N // 2, 2, P, KTP_N, 2, P).transpose(0, 5, 1, 3, 4, 2)
    )

    # w2 packed [p, u, cols]: per covered (ob, j) in (ob asc, j asc) order
    chunks = []
    for ob in range(OB_N):
        for j in sel[ob]:
            blk = w2[ob * P : (ob + 1) * P, j * 2 * P : (j + 1) * 2 * P]
            chunks.append(blk.reshape(P, 2, P).transpose(2, 1, 0))
    if chunks:
        w2t = np.ascontiguousarray(np.concatenate(chunks, axis=2))
    else:
        w2t = np.zeros((P, 2, P), dtype=ml_dtypes.float8_e4m3)

    in_maps = []
    for c in range(N_CORES):
        msl = slice(c * M_LOC, (c + 1) * M_LOC)
        # xs[p, j, u, m]: k = j*256 + u*128 + p
        xqc = np.ascontiguousarray(
            xq_all[msl].T.reshape(KTP_N, 2, P, M_LOC).transpose(2, 0, 1, 3)
        )
        scc = np.ascontiguousarray(
            np.broadcast_to(
                (t / HALF_MAX * amax[msl]).astype(np.float32)[None, :], (P, M_LOC)
            )
        )
        in_maps.append({"xq": xqc, "w1t": w1t, "w2t": w2t, "scb": scc})
    return in_maps, sel


def assemble_output(results, x):
    y = np.empty((M_FULL, O_FULL), dtype=np.float32)
    for c in range(N_CORES):
        y[c * M_LOC : (c + 1) * M_LOC] = results[c]["yt"].T
    return y.reshape(*x.shape[:-1], O_FULL)


_NC_CACHE = {}


def run(x, weight, weight_scale_inv, trace=False):
    from concourse.bass_utils import run_bass_kernel_spmd

    in_maps, sel = prep_inputs(
        np.asarray(x), np.asarray(weight), np.asarray(weight_scale_inv)
    )
    key = tuple(tuple(s) for s in sel)
    if key not in _NC_CACHE:
        nc_new = build_bass(sel)
        nc_new.finalize()
        _NC_CACHE.clear()
        _NC_CACHE[key] = nc_new
    nc = _NC_CACHE[key]
    res = run_bass_kernel_spmd(
        nc, in_maps, core_ids=list(range(N_CORES)), trace=trace
    )
    y = assemble_output(res.results, np.asarray(x))
    return y, res


def kernel(x, weight, weight_scale_inv):
    y, _ = run(x, weight, weight_scale_inv)
    return y


# revision 6
# speedup vs baseline: 1.0106x; 1.0106x over previous
"""Block-scaled fp8 ColumnParallelLinear for Trainium2 — w-stationary fp8
DoubleRow GEMM with exact top-N partial-residual correction.
Measured: ~296us, rel err 1.942e-2 (baseline 350.7us / 1.447e-2).

Reference semantics (per token m, output o):
    x_scale[m] = max(|x[m, :]|) / 448
    x_q[m, k]  = e4m3fn_round(x[m, k] / x_scale[m])
    w_deq[o,k] = e4m3(w)[o, k] * s[o//128, k//128]
    y[m, o]    = x_scale[m] * sum_k x_q[m, k] * w_deq[o, k]

Approach (8 M-shards, O replicated -> one SPMD program):
  - Host does ALL quantization. x on the exact half grid (224/amax,
    TRN e4m3 max 240): device bytes = reference fp8 values / 2, the
    factor folds into the output scale sc[m] = t * amax[m]/224, so the
    x side contributes ZERO error vs the reference.
  - w1 = trn_fp8(w_deq/t) with one global t; residual w2 =
    trn_fp8(w_deq/t - w1) applied only on the top-N_COVER (128o x
    256k) blocks by residual energy (N=248 of 1024 -> rel err
    1.942e-2, gate 2e-2; host model matches hw to <0.1% across six
    configs) at 1.24x the pure-fp8 FLOPs. 1272 matmuls x 216ns =
    the 274.8us tensor floor.
  - Device: W-STATIONARY fp8 DoubleRow matmuls. Stationary =
    w tile [128k, 2, 128o]; moving = x [128k, 2, 512m] (the whole
    M-shard per instr; moving max 1024 = 2*512). Out psum
    [128o, 512m] = 1 bank. Every matmul (main or residual) is a full
    512-col instr (213ns) > LDWEIGHTS (135ns), so the per-matmul
    LDWEIGHTS that legalization always inserts stays hidden, and the
    residual needs no pair-merging/padding: exactly the selected
    blocks are covered, one instr each.
  - Pair-interleaved compute: the two o-blocks of an ob-pair sweep j
    together (two live psum banks), so each x j-chunk feeds two
    matmuls on arrival — halves the startup bandwidth demand. Per ob:
    16 main DR matmuls + one per covered (ob, j); residuals sit at
    the end of the sweep (w2 off the critical DMA prefix). DVE
    applies the per-token scale (free-dim vector, broadcast across
    partitions); output DMA (scalar engine) writes yt[o, m]
    transposed, host transposes back. Last ob drains in halves.
  - One sync-engine DMA FIFO in exact need order (same-queue FIFO =
    implicit bandwidth priority; parallel queues contend): 4-j
    startup chunks for the lightest-residual pair first, then 1MiB
    per-pair slabs, 5-deep rotation. dma_start costs ~0.7us of issue
    time each, so startup uses few, large transfers.
  - 28 warm-up matmuls on zeroed scratch ramp the PE clock out of its
    low p-state during the ~5us startup DMA wait; without them the
    first ~10 real matmuls run ~2x slow.
"""

import numpy as np
import ml_dtypes

import concourse.bass as bass
import concourse.mybir as mybir
from concourse import bacc
from concourse.tile import TileContext

FP8_MAX = 448.0
HALF_MAX = 224.0
P = 128
BLOCK = 128

M_FULL, K_FULL, O_FULL = 4096, 4096, 8192
N_CORES = 8
M_LOC = M_FULL // N_CORES  # 512 tokens per core; O replicated

KTP_N = K_FULL // (2 * P)  # 16 k-pair tiles
OB_N = O_FULL // P  # 64 o-blocks
N_COVER = 248  # top-N residual blocks of OB_N*KTP_N=1024

DR = mybir.MatmulPerfMode.DoubleRow


def _select(resid):
    """sel[ob] = ascending list of covered j for the top-N_COVER blocks."""
    E = (resid.astype(np.float32) ** 2).reshape(OB_N, P, KTP_N, 2 * P).sum(axis=(1, 3))
    flat = E.flatten()
    order = np.argsort(flat)[::-1][:N_COVER]
    mask = np.zeros(flat.size, bool)
    mask[order] = True
    mask = mask.reshape(OB_N, KTP_N)
    return [sorted(np.nonzero(mask[ob])[0].tolist()) for ob in range(OB_N)]


def build_bass(sel):
    nc = bacc.Bacc()
    f32 = mybir.dt.float32
    fp8 = mybir.dt.float8e4

    nsel = [len(s) for s in sel]
    total_cols = sum(nsel) * P

    xq = nc.declare_dram_parameter("xq", [P, KTP_N, 2, M_LOC], fp8, isOutput=False)
    # pair-major: [pair, p, half, j, u, ol] so one pair DMA is contiguous
    w1t = nc.declare_dram_parameter(
        "w1t", [OB_N // 2, P, 2, KTP_N, 2, P], fp8, isOutput=False
    )
    w2t = nc.declare_dram_parameter(
        "w2t", [P, 2, max(total_cols, P)], fp8, isOutput=False
    )
    scb = nc.declare_dram_parameter("scb", [P, M_LOC], f32, isOutput=False)
    yt = nc.declare_dram_parameter("yt", [O_FULL, M_LOC], f32, isOutput=True)

    # w2 column offset (in o-blocks) per ob, in (ob, j) packing order
    off = np.cumsum([0] + nsel)

    with TileContext(nc) as tc:
        with (
            tc.tile_pool(name="xp", bufs=1) as xp,
            tc.tile_pool(name="wp", bufs=32) as wp,
            tc.tile_pool(name="w2p", bufs=32) as w2p,
            tc.tile_pool(name="op", bufs=6) as op,
            tc.tile_pool(name="mm", bufs=8, space="PSUM") as pp,
        ):
            xs = xp.tile([P, KTP_N, 2, M_LOC], fp8)
            scs = xp.tile([P, M_LOC], f32)

            # Warm-up: dependency-free matmuls on (uninitialized) scratch
            # start right after the tensor preamble, ramping the PE clock
            # through the ~5us DMA wait so real matmuls run at full speed
            # from the first one. Results land in an otherwise-unused psum
            # bank and are never read.
            warm = xp.tile([P, 2, P], fp8)
            psw = pp.tile([P, M_LOC], f32, tag="warm", bufs=1, name="ps_warm")
            nc.scalar.memzero(warm[:])
            for i in range(28):
                nc.tensor.matmul(
                    psw[:, 0:P],
                    warm[:],
                    warm[:],
                    start=True,
                    stop=True,
                    perf_mode=DR,
                )

            OBP = OB_N // 2  # 32 ob-pairs; one w1/w2 DMA per pair
            # process the lightest-residual pair first: smallest critical
            # startup prefix (its w2 slab may even be empty)
            p0 = min(range(OBP), key=lambda pr: nsel[2 * pr] + nsel[2 * pr + 1])
            pair_order = [p0] + [pr for pr in range(OBP) if pr != p0]
            wtiles = {}
            rtiles = {}
            for pr in pair_order:
                wtiles[pr] = wp.tile(
                    [P, 2, KTP_N, 2, P], fp8, tag="w1", bufs=5, name=f"w1_{pr}"
                )
                pw = (nsel[2 * pr] + nsel[2 * pr + 1]) * P
                rtiles[pr] = (
                    w2p.tile([P, 2, pw], fp8, tag="w2", bufs=5, name=f"w2_{pr}")
                    if pw
                    else None
                )

            def issue_pair_dma(pr):
                nc.sync.dma_start(out=wtiles[pr][:], in_=w1t[pr])
                if rtiles[pr] is not None:
                    c0, c1 = off[2 * pr] * P, off[2 * pr + 2] * P
                    nc.sync.dma_start(
                        out=rtiles[pr][:], in_=w2t[:, :, c0:c1]
                    )

            # Startup: one sync FIFO stream in exact need order (same-queue
            # FIFO = implicit bandwidth priority; parallel queues contend).
            # Pair 0 is computed ob-interleaved per j, so each 4-j chunk
            # (x 512KB + both-halves w1 256KB) feeds 8 matmuls — delivery
            # and consumption stay matched from the first matmul on.
            for c in range(0, KTP_N, 4):
                nc.sync.dma_start(out=xs[:, c : c + 4], in_=xq[:, c : c + 4])
                nc.sync.dma_start(
                    out=wtiles[p0][:, :, c : c + 4], in_=w1t[p0, :, :, c : c + 4]
                )
            if rtiles[p0] is not None:
                c0, c1 = off[2 * p0] * P, off[2 * p0 + 2] * P
                nc.sync.dma_start(out=rtiles[p0][:], in_=w2t[:, :, c0:c1])
            issue_pair_dma(pair_order[1])
            nc.sync.dma_start(out=scs[:], in_=scb[:])
            for pr in pair_order[2:]:
                issue_pair_dma(pr)

            # Pair-interleaved compute: the two obs of a pair sweep j
            # together (two live psum banks), so each x j-chunk feeds two
            # matmuls as soon as it lands. Per ob: residuals, then the
            # stop=True j15 main; the even ob's DVE+output drain overlaps
            # the odd ob's residual tail.
            for pr in pair_order:
                wt = wtiles[pr]
                obs = (2 * pr, 2 * pr + 1)
                pss = [
                    pp.tile([P, M_LOC], f32, tag="ps", bufs=6, name=f"ps_{ob}")
                    for ob in obs
                ]
                for j in range(KTP_N - 1):
                    for half, ps in enumerate(pss):
                        nc.tensor.matmul(
                            ps[:],
                            wt[:, half, j],
                            xs[:, j],
                            start=(j == 0),
                            stop=False,
                            perf_mode=DR,
                        )
                for half, ps in enumerate(pss):
                    ob = obs[half]
                    ridx = off[ob] - off[2 * pr]
                    for j in sel[ob]:
                        nc.tensor.matmul(
                            ps[:],
                            rtiles[pr][:, :, ridx * P : (ridx + 1) * P],
                            xs[:, j],
                            start=False,
                            stop=False,
                            perf_mode=DR,
                        )
                        ridx += 1
                    nc.tensor.matmul(
                        ps[:],
                        wt[:, half, KTP_N - 1],
                        xs[:, KTP_N - 1],
                        start=False,
                        stop=True,
                        perf_mode=DR,
                    )
                    ot = op.tile([P, M_LOC], f32, tag="out", bufs=4, name=f"o_{ob}")
                    if ob == 2 * pair_order[-1] + 1:
                        # last ob: halve the DVE+DMA drain so the tail
                        # pipelines (DVE h2 overlaps DMA h1)
                        H = M_LOC // 2
                        for h in range(2):
                            sl = slice(h * H, (h + 1) * H)
                            nc.vector.tensor_tensor(
                                ot[:, sl], ps[:, sl], scs[:, sl],
                                mybir.AluOpType.mult,
                            )
                            nc.scalar.dma_start(
                                out=yt[ob * P : (ob + 1) * P, sl], in_=ot[:, sl]
                            )
                    else:
                        nc.vector.tensor_tensor(
                            ot[:], ps[:], scs[:], mybir.AluOpType.mult
                        )
                        nc.scalar.dma_start(
                            out=yt[ob * P : (ob + 1) * P, :], in_=ot[:]
                        )
    return nc


def prep_inputs(x, weight, weight_scale_inv):
    x2d = np.ascontiguousarray(x.reshape(M_FULL, K_FULL).astype(np.float32))
    amax = np.clip(np.abs(x2d).max(axis=1), 1e-12, None)
    xq_all = (x2d * (HALF_MAX / amax)[:, None]).astype(ml_dtypes.float8_e4m3)

    w8 = weight.astype(ml_dtypes.float8_e4m3fn).astype(np.float32)
    s_exp = np.repeat(
        np.repeat(weight_scale_inv.astype(np.float32), BLOCK, 0), BLOCK, 1
    )
    w_deq = w8 * s_exp
    t = float(np.abs(w_deq).max() / HALF_MAX)
    v = w_deq / t
    w1 = v.astype(ml_dtypes.float8_e4m3)
    resid = v - w1.astype(np.float32)
    w2 = resid.astype(ml_dtypes.float8_e4m3)
    sel = _select(resid)

    # w1t[pr, p, half, j, u, ol] from w1[o, k]:
    #   o = pr*256 + half*128 + ol, k = j*256 + u*128 + p
    w1t = np.ascontiguousarray(
        w1.reshape(OB_N // 2, 2, P, KTP_N, 2, P).transpose(0, 5, 1, 3, 4, 2)
    )

    # w2 packed [p, u, cols]: per covered (ob, j) in (ob asc, j asc) order
    chunks = []
    for ob in range(OB_N):
        for j in sel[ob]:
            blk = w2[ob * P : (ob + 1) * P, j * 2 * P : (j + 1) * 2 * P]
            chunks.append(blk.reshape(P, 2, P).transpose(2, 1, 0))
    if chunks:
        w2t = np.ascontiguousarray(np.concatenate(chunks, axis=2))
    else:
        w2t = np.zeros((P, 2, P), dtype=ml_dtypes.float8_e4m3)

    in_maps = []
    for c in range(N_CORES):
        msl = slice(c * M_LOC, (c + 1) * M_LOC)
        # xs[p, j, u, m]: k = j*256 + u*128 + p
        xqc = np.ascontiguousarray(
            xq_all[msl].T.reshape(KTP_N, 2, P, M_LOC).transpose(2, 0, 1, 3)
        )
        scc = np.ascontiguousarray(
            np.broadcast_to(
                (t / HALF_MAX * amax[msl]).astype(np.float32)[None, :], (P, M_LOC)
            )
        )
        in_maps.append({"xq": xqc, "w1t": w1t, "w2t": w2t, "scb": scc})
    return in_maps, sel


def assemble_output(results, x):
    y = np.empty((M_FULL, O_FULL), dtype=np.float32)
    for c in range(N_CORES):
        y[c * M_LOC : (c + 1) * M_LOC] = results[c]["yt"].T
    return y.reshape(*x.shape[:-1], O_FULL)


_NC_CACHE = {}


def run(x, weight, weight_scale_inv, trace=False):
    from concourse.bass_utils import run_bass_kernel_spmd

    in_maps, sel = prep_inputs(
        np.asarray(x), np.asarray(weight), np.asarray(weight_scale_inv)
    )
    key = tuple(tuple(s) for s in sel)
    if key not in _NC_CACHE:
        nc_new = build_bass(sel)
        nc_new.finalize()
        _NC_CACHE.clear()
        _NC_CACHE[key] = nc_new
    nc = _NC_CACHE[key]
    res = run_bass_kernel_spmd(
        nc, in_maps, core_ids=list(range(N_CORES)), trace=trace
    )
    y = assemble_output(res.results, np.asarray(x))
    return y, res


def kernel(x, weight, weight_scale_inv):
    y, _ = run(x, weight, weight_scale_inv)
    return y
